# revision 31
# baseline (speedup 1.0000x reference)
"""Trainium2 Bass kernel for nn_BetaVAEMark7Decoder (v2).

All six layers are matmuls on the TensorEngine; conv pairs are fused on the
host into banded composite blocks (up1*tc1, up2*tc2, up3*tc3). Data-parallel
over batch: 4096 rows split 512 per core.

v2 structural changes vs the 406us baseline:
- fused3 runs as stationary-reuse streams: per (batch-chunk, j-window) the
  a3 activation slice for input row i is loaded once and fires 1-2 merged
  matmuls into a rolling 2-slot-per-bank PSUM ring, relying on PSUM
  has_written semantics (accumulate where written, overwrite where not).
- j-windows (0,9),(4,13),(12,13),(20,12) with 8-wide ownership; window rows
  are permuted so owned rows sit at [0:64) making every fused2 evacuation a
  single full-width [64,512] instruction; halo rows filled by SBUF DMAs.
- biases folded into the matmuls via ones-rows (x3 and a3) so all evacs are
  single-pass lrelu, round-robined across Scalar and Vector engines.
- output staged in bf16 (c4-major, 4 h-rows per tile -> 512B descriptors),
  upcast to f32 on the host.
"""
import numpy as np
from contextlib import ExitStack

import concourse.bass as bass
import concourse.tile as tile
from concourse import bacc, mybir
from concourse.bass_utils import run_bass_kernel_spmd

F32 = mybir.dt.float32
F32R = mybir.dt.float32r
BF16 = mybir.dt.bfloat16
AF = mybir.ActivationFunctionType
OP = mybir.AluOpType

NCORES = 8
BCORE = 512

# fused3 (a3) j-windows over j=W2 in [0,32): (j0, nj); window t owns j in [8t, 8t+8)
F3_WIN = [(0, 9), (4, 13), (12, 13), (20, 12)]
# fused2 input (x3) windows over j=W1 in [0,16): (j0, nj); window a primary j in [4a, 4a+4)
X2_WIN = [(0, 5), (2, 7), (6, 7), (10, 6)]
# fused1 input (x1) windows over wi in [0,8)
X1_WIN = [(0, 3), (1, 4), (3, 4), (5, 3)]
HG = [(0, 2), (2, 2), (4, 1)]


def _x3_row(a, j):
    """Row base (of 16) for x2-col j in x3 window a: primary [0:64), halos after."""
    j0, nj = X2_WIN[a]
    p0 = 4 * a
    if p0 <= j < p0 + 4:
        return (j - p0) * 16
    if j < p0:
        return 64 + (j - j0) * 16
    return 64 + (p0 - j0) * 16 + (j - (p0 + 4)) * 16


def _x3_ones(a):
    return X2_WIN[a][1] * 16


def _a3_row(t, j):
    """Row base (of 8) for W2-col j in a3 window t: owned [0:64), halos after."""
    j0, nj = F3_WIN[t]
    p0 = 8 * t
    if p0 <= j < p0 + 8:
        return (j - p0) * 8
    if j < p0:
        return 64 + (j - j0) * 8
    return 64 + (p0 - j0) * 8 + (j - (p0 + 8)) * 8


def _a3_ones(t):
    return F3_WIN[t][1] * 8


# ---------------- host-side weight factorization ----------------
def _precompute(w):
    P = {}
    w_lin, b_lin = w["w_lin"], w["b_lin"]
    lhs_lin = np.zeros((7, 256), np.float32)
    c_lin = np.zeros(256, np.float32)
    for wi in range(8):
        for ci in range(32):
            lhs_lin[:, wi * 32 + ci] = w_lin[:, ci * 8 + wi]
            c_lin[wi * 32 + ci] = b_lin[ci * 8 + wi]
    P["lhs_lin"], P["c_lin"] = lhs_lin, c_lin

    w_up1, b_up1, w_tc1, b_tc1 = w["w_up1"], w["b_up1"], w["w_tc1"], w["b_tc1"]
    K1 = np.zeros((5, 2, 3, 32, 16), np.float32)
    for hh in range(5):
        for s in range(2):
            for dh in range(3):
                hp = hh + 1 - dh
                if not (0 <= hp < 5):
                    continue
                for dw in range(3):
                    t = s + 1 - dw
                    dj = int(np.floor(t / 2))
                    kw = t - 2 * dj
                    K1[hh, s, dj + 1] += np.einsum("ic,cd->id", w_up1[hp, kw], w_tc1[dh, dw])
    c1 = np.zeros((5, 16, 16), np.float32)
    for hh in range(5):
        for ww in range(16):
            acc = b_tc1.copy()
            for dh in range(3):
                if not (0 <= hh + 1 - dh < 5):
                    continue
                for dw in range(3):
                    if not (0 <= ww + 1 - dw < 16):
                        continue
                    acc = acc + b_up1 @ w_tc1[dh, dw]
            c1[hh, ww] = acc
    P["K1"], P["c1"] = K1, c1

    w_up2, b_up2, w_tc2, b_tc2 = w["w_up2"], w["b_up2"], w["w_tc2"], w["b_tc2"]
    K2 = np.zeros((5, 2, 3, 3, 16, 8), np.float32)
    for r in range(5):
        for s in range(2):
            for dh in range(3):
                u = r + 1 - dh
                di = int(np.floor(u / 5))
                kh = u - 5 * di
                for dw in range(3):
                    t = s + 1 - dw
                    dj = int(np.floor(t / 2))
                    kw = t - 2 * dj
                    K2[r, s, di + 1, dj + 1] += np.einsum("ic,cd->id", w_up2[kh, kw], w_tc2[dh, dw])
    P["K2"] = K2
    P["BB2"] = np.einsum("c,hwcd->hwd", b_up2, w_tc2)
    P["b_tc2"] = b_tc2

    w_up3, b_up3, w_tc3, b_tc3 = w["w_up3"], w["b_up3"], w["w_tc3"], w["b_tc3"]
    K3 = np.zeros((2, 2, 3, 3, 8, 6), np.float32)
    for r in range(2):
        for s in range(2):
            for dh in range(3):
                u = r + 1 - dh
                di = int(np.floor(u / 2))
                kh = u - 2 * di
                for dw in range(3):
                    t = s + 1 - dw
                    dj = int(np.floor(t / 2))
                    kw = t - 2 * dj
                    K3[r, s, di + 1, dj + 1] += np.einsum("ic,cd->id", w_up3[kh, kw], w_tc3[dh, dw])
    P["K3"] = K3
    P["BB3"] = np.einsum("c,hwcd->hwd", b_up3, w_tc3)
    P["b_tc3"] = b_tc3
    return P


def _fused1_blocks(P):
    K1 = P["K1"]
    blocks, biases = {}, {}
    for g, (h0, nh) in enumerate(HG):
        for a in range(4):
            wi0, nwi = X1_WIN[a]
            M = nh * 4 * 16
            B = np.zeros((nwi * 32, M), np.float32)
            bias = np.zeros(M, np.float32)
            for hi in range(nh):
                hh = h0 + hi
                for wl in range(4):
                    ww = 4 * a + wl
                    j, s = ww // 2, ww % 2
                    for c2 in range(16):
                        col = hi * 64 + wl * 16 + c2
                        bias[col] = P["c1"][hh, ww, c2]
                        for wi_l in range(nwi):
                            dj = (wi0 + wi_l) - j
                            if -1 <= dj <= 1:
                                B[wi_l * 32:(wi_l + 1) * 32, col] = K1[hh, s, dj + 1, :, c2]
            blocks[(g, a)] = B
            biases[(g, a)] = bias
    return blocks, biases


def _f2_col_bias(P, Hh, Ww, c3):
    acc = P["b_tc2"][c3]
    for dh in range(3):
        if not (0 <= Hh + 1 - dh < 25):
            continue
        for dw in range(3):
            if not (0 <= Ww + 1 - dw < 32):
                continue
            acc += P["BB2"][dh, dw, c3]
    return acc


def _fused2_blocks(P):
    """Blocks with x3 row permutation and bias rows at the ones-row position."""
    K2 = P["K2"]
    blocks = {}
    for a in range(4):
        j0, nj = X2_WIN[a]
        K = nj * 16 + 1
        ones = _x3_ones(a)

        def fill(B, colbase, r, di, bias_i=None):
            for wl in range(8):
                Ww = 8 * a + wl
                j, s = Ww // 2, Ww % 2
                for c3 in range(8):
                    col = colbase + wl * 8 + c3
                    for j2 in range(j0, j0 + nj):
                        dj = j2 - j
                        if -1 <= dj <= 1:
                            rb = _x3_row(a, j2)
                            B[rb:rb + 16, col] = K2[r, s, di + 1, dj + 1, :, c3]
                    if bias_i is not None:
                        B[ones, col] = _f2_col_bias(P, 5 * bias_i + r, Ww, c3)

        B = np.zeros((K, 128), np.float32)
        fill(B, 0, 1, 0, bias_i=1)
        fill(B, 64, 2, 0, bias_i=1)
        blocks[("r12", a)] = B
        for tag, bi in (("mid", 2), ("edge", 0)):
            B = np.zeros((K, 128), np.float32)
            fill(B, 0, 0, 0, bias_i=bi)
            fill(B, 64, 3, 0, bias_i=1)
            blocks[("m", tag, a)] = B
        B = np.zeros((K, 64), np.float32)
        fill(B, 0, 0, -1)
        blocks[("r0m1", a)] = B
        for tag, bi in (("mid", 2), ("edge", 4)):
            B = np.zeros((K, 64), np.float32)
            fill(B, 0, 4, 0, bias_i=bi)
            blocks[("r4", tag, a)] = B
        B = np.zeros((K, 64), np.float32)
        fill(B, 0, 4, 1)
        blocks[("r4p1", a)] = B
    return blocks


def _fused3_blocks(P):
    """Per t: cat [K,384] = [W(+1)r1 | W(0) | W(-1)r0], e0 [K,288], e24 [K,192].
    Slot col order r*96 + c4*16 + (jc-8t)*2 + s; a3 row permutation applied."""
    K3, BB3, b_tc3 = P["K3"], P["BB3"], P["b_tc3"]
    blocks = {}
    for t in range(4):
        j0, nj = F3_WIN[t]
        K = nj * 8 + 1
        ones = _a3_ones(t)

        def w_block(di, rsel, iclass=None):
            B = np.zeros((K, len(rsel) * 96), np.float32)
            for ri, r in enumerate(rsel):
                for c4 in range(6):
                    for jc in range(8 * t, 8 * t + 8):
                        for s in range(2):
                            col = ri * 96 + c4 * 16 + (jc - 8 * t) * 2 + s
                            for j2 in range(j0, j0 + nj):
                                dj = j2 - jc
                                if -1 <= dj <= 1:
                                    rb = _a3_row(t, j2)
                                    B[rb:rb + 8, col] = K3[r, s, di + 1, dj + 1, :, c4]
                            if iclass is not None and di == 0:
                                acc = b_tc3[c4]
                                for dh in range(3):
                                    u = r + 1 - dh
                                    di_ = int(np.floor(u / 2))
                                    ok = (iclass == 0) or (iclass == 1 and di_ >= 0) \
                                        or (iclass == 2 and di_ <= 0)
                                    if not ok:
                                        continue
                                    for dw in range(3):
                                        tt = s + 1 - dw
                                        dj_ = int(np.floor(tt / 2))
                                        if 0 <= jc + dj_ < 32:
                                            acc += BB3[dh, dw, c4]
                                B[ones, col] = acc
            return B

        w1r1 = w_block(1, [1])
        wm1r0 = w_block(-1, [0])
        blocks[("cat", t)] = np.concatenate([w1r1, w_block(0, [0, 1], 0), wm1r0], axis=1)
        blocks[("e0", t)] = np.concatenate([w_block(0, [0, 1], 1), wm1r0], axis=1)
        blocks[("e24", t)] = w_block(0, [0, 1], 2)
    return blocks


class _Pack:
    def __init__(self):
        self.cols = 0
        self.reg = {}
        self.items = []

    def add(self, key, arr):
        K, M = arr.shape
        self.reg[key] = (self.cols, K, M)
        self.items.append(arr)
        self.cols += M

    def build(self):
        out = np.zeros((128, self.cols), np.float32)
        c = 0
        for arr in self.items:
            K, M = arr.shape
            out[:K, c:c + M] = arr
            c += M
        return out


def _make_packs(inputs):
    P = _precompute(inputs)
    f1b, f1bias = _fused1_blocks(P)
    f2b = _fused2_blocks(P)
    f3b = _fused3_blocks(P)

    # order: lin + bias rows first so the first two 512-col chunks unblock lin
    wp = _Pack()
    wp.add("lin0", P["lhs_lin"][:, 0:128])
    wp.add("lin1", P["lhs_lin"][:, 128:256])
    # row-form biases + a ones row: biases enter PSUM via a K=1 matmul
    wp.add("ones512", np.ones((1, BCORE), np.float32))
    wp.add("rblin0", P["c_lin"][0:128].reshape(1, -1))
    wp.add("rblin1", P["c_lin"][128:256].reshape(1, -1))
    for g in range(3):
        for a in range(4):
            wp.add(("rb1", g, a), f1bias[(g, a)].reshape(1, -1))
    for g in range(3):
        for a in range(4):
            wp.add(("f1", g, a), f1b[(g, a)])

    wb = _Pack()
    for a in range(4):
        for key in [("r12", a), ("m", "mid", a), ("m", "edge", a), ("r0m1", a),
                    ("r4", "mid", a), ("r4", "edge", a), ("r4p1", a)]:
            wb.add(key, f2b[key])
    for t in range(4):
        for key in [("cat", t), ("e0", t), ("e24", t)]:
            wb.add(key, f3b[key])

    bp = _Pack()
    bp.add("blin0", P["c_lin"][0:128].reshape(-1, 1))
    bp.add("blin1", P["c_lin"][128:256].reshape(-1, 1))
    for g in range(3):
        for a in range(4):
            bp.add(("b1", g, a), f1bias[(g, a)].reshape(-1, 1))
    return wp, bp, wb


# ---------------- device program ----------------
_PROG = {}


def _lim(s):
    if s == 0:
        return 128
    if s == 64:
        return 64
    return 32


def _pieces(p0, d0, n):
    assert p0 % 32 == 0 and d0 % 32 == 0, (p0, d0, n)
    out = []
    off = 0
    while off < n:
        s1, s2 = (p0 + off) % 128, (d0 + off) % 128
        c = min(n - off, _lim(s1), _lim(s2))
        out.append((off, c))
        off += c
    return out


def _build_program(wcols, bcols, wbcols):
    key = (wcols, bcols, wbcols)
    if key in _PROG:
        return _PROG[key]
    nc = bacc.Bacc("TRN2", target_bir_lowering=False, debug=False, num_devices=NCORES)
    lat_ap = nc.dram_tensor("latent", [BCORE, 7], F32, kind="ExternalInput").ap()
    wp_ap = nc.dram_tensor("wpack", [128, wcols], F32, kind="ExternalInput").ap()
    bp_ap = nc.dram_tensor("bpack", [128, bcols], F32, kind="ExternalInput").ap()
    wb_ap = nc.dram_tensor("wbpack", [128, wbcols], BF16, kind="ExternalInput").ap()
    # h-major output (host transposes to NCHW): fully contiguous stg DMA
    out_ap = nc.dram_tensor("out", [BCORE, 50, 6, 64], BF16, kind="ExternalOutput").ap()
    ones_ap = nc.dram_tensor("ones", [1, 25 * BCORE], BF16, kind="ExternalInput").ap()
    with tile.TileContext(nc) as tc:
        with ExitStack() as ctx:
            _emit(ctx, tc, nc, lat_ap, wp_ap, bp_ap, wb_ap, out_ap, ones_ap,
                  _build_program.wreg, _build_program.breg, _build_program.wbreg)
    _dedup_ldweights(nc)
    nc.compile()
    _PROG[key] = nc
    return nc


def _dedup_ldweights(nc):
    """Drop InstLdweights whose stationary AP matches the previous load on the
    PE queue (the PE array keeps the stationary across matmuls)."""
    from concourse import mybir
    removed = 0
    for fn in nc.m.functions:
        for blk in fn.blocks:
            insts = list(blk.instructions)
            keep = []
            prev_sig = None
            for ins in insts:
                tn = type(ins).__name__
                if tn == "InstLdweights":
                    sig = (str(ins.ins[0]), str(getattr(ins, "perf_mode", None)),
                           str(getattr(ins, "is_transpose", None)))
                    if sig == prev_sig and not ins.has_wait() and not ins.has_update():
                        removed += 1
                        continue
                    prev_sig = sig
                elif tn == "InstMatmult":
                    pass  # streaming doesn't clobber the loaded stationary
                elif getattr(ins, "engine", None) == mybir.EngineType.PE \
                        and tn not in ("InstEventSemaphore",):
                    prev_sig = None
                keep.append(ins)
            if len(keep) != len(insts):
                blk.instructions = keep
    return removed


def _emit(ctx, tc, nc, lat_ap, wp_ap, bp_ap, wb_ap, out_ap, ones_ap, wreg, breg, wbreg):
    wcols = wp_ap.shape[1]
    bcols = bp_ap.shape[1]
    wbcols = wb_ap.shape[1]

    consts = ctx.enter_context(tc.tile_pool(name="consts", bufs=1))
    bounce = ctx.enter_context(tc.tile_pool(name="bounce", bufs=2))
    x1p = ctx.enter_context(tc.tile_pool(name="x1", bufs=1))
    x3p = ctx.enter_context(tc.tile_pool(name="x3", bufs=1))
    a3p = ctx.enter_context(tc.tile_pool(name="a3", bufs=1))
    stgp = ctx.enter_context(tc.tile_pool(name="stg", bufs=7))
    psp = ctx.enter_context(tc.tile_pool(name="ps", bufs=4, space="PSUM"))

    # ---- constants (lin dependencies first, big fused-weight pack last) ----
    lat_f = consts.tile([7, BCORE], F32)
    nc.sync.dma_start(lat_f[:], lat_ap[:].rearrange("b d -> d b"))
    lat_r = consts.tile([7, BCORE], F32R)
    nc.vector.tensor_copy(lat_r[:], lat_f[:])
    wp_r = consts.tile([128, wcols], F32R)
    for c0 in range(0, wcols, 512):
        n = min(512, wcols - c0)
        bt = bounce.tile([128, 512], F32, tag="bounce", name=f"bw{c0}")
        nc.sync.dma_start(bt[:, :n], wp_ap[:, c0:c0 + n])
        nc.vector.tensor_copy(wp_r[:, c0:c0 + n], bt[:, :n])
    wbt = consts.tile([128, wbcols], BF16)
    nc.sync.dma_start(wbt[:], wb_ap[:])
    bpt = consts.tile([128, bcols], F32)
    nc.sync.dma_start(bpt[:], bp_ap[:])

    def W(key):
        o, K, M = wreg[key]
        return wp_r[:K, o:o + M]

    def WB(key, c0=None, c1=None):
        o, K, M = wbreg[key]
        if c0 is None:
            return wbt[:K, o:o + M]
        return wbt[:K, o + c0:o + c1]

    def BV(key, p0, n):
        o, K, M = breg[key]
        return bpt[p0:p0 + n, o:o + 1]

    # evac engines: ACT does lrelu via activation, DVE via scalar_tensor_tensor
    ev_ctr = [0]

    def ev_lrelu(dst, src):
        # DVE cannot read two PSUM operands in one instruction (and Pool
        # rejects stt entirely): DVE path = PSUM->SBUF copy + in-place lrelu.
        # ACT single-pass is ~2x cheaper, so it takes 2 of every 3.
        if ev_ctr[0] % 3 != 2:
            nc.scalar.activation(dst, src, AF.Lrelu, bias=0.0, scale=1.0, alpha=0.01)
        else:
            nc.vector.tensor_copy(dst, src)
            nc.vector.scalar_tensor_tensor(dst, dst, 0.01, dst, op0=OP.mult, op1=OP.max)
        ev_ctr[0] += 1

    # ---- x3 / a3 tiles + ones rows ----
    x3t = [x3p.tile([X2_WIN[a][1] * 16 + 1, 5 * BCORE], BF16, tag=f"x3_{a}",
                    name=f"x3_{a}") for a in range(4)]
    a3t = [a3p.tile([F3_WIN[t][1] * 8 + 1, 25 * BCORE], BF16, tag=f"a3_{t}",
                    name=f"a3_{t}") for t in range(4)]
    # ones rows via DMA from a DRAM constant (gpsimd memset is ~10us per row)
    for a in range(4):
        o = _x3_ones(a)
        nc.sync.dma_start(x3t[a][o:o + 1, :], ones_ap[0:1, 0:5 * BCORE])
    for t in range(4):
        o = _a3_ones(t)
        nc.sync.dma_start(a3t[t][o:o + 1, :], ones_ap[0:1, :])

    # ---- lin (bias preloaded into PSUM via K=1 matmul against a ones row) ----
    psL = psp.tile([128, 2 * BCORE], F32, tag="ps", name="lin")
    psA, psB = psL[:, 0:BCORE], psL[:, BCORE:2 * BCORE]
    nc.tensor.matmul(psA, W("rblin0"), W("ones512"), start=True, stop=False,
                     skip_group_check=True)
    nc.tensor.matmul(psA, W("lin0"), lat_r[:], start=False, stop=True,
                     skip_group_check=True)
    nc.tensor.matmul(psB, W("rblin1"), W("ones512"), start=True, stop=False,
                     skip_group_check=True)
    nc.tensor.matmul(psB, W("lin1"), lat_r[:], start=False, stop=True,
                     skip_group_check=True)

    x1t = [x1p.tile([X1_WIN[a][1] * 32, BCORE], F32R, tag=f"x1_{a}", name=f"x1_{a}")
           for a in range(4)]
    for a in range(4):
        wi0, nwi = X1_WIN[a]
        for ps, base in ((psA, 0), (psB, 4)):
            lo = max(wi0, base)
            hi = min(wi0 + nwi, base + 4)
            if lo >= hi:
                continue
            d0 = (lo - wi0) * 32
            p0 = (lo - base) * 32
            n = (hi - lo) * 32
            for off, cnt in _pieces(p0, d0, n):
                ev_lrelu(x1t[a][d0 + off:d0 + off + cnt, :],
                         ps[p0 + off:p0 + off + cnt, :])

    # ---- fused1 (a-outer so x3 mirrors can fire early) ----
    # x3 mirrors: (dst_a, d0, src_a, s0, n)
    X3_MIR = [(0, 64, 1, 0, 16), (1, 64, 0, 32, 32), (1, 96, 2, 0, 16),
              (2, 64, 1, 32, 32), (2, 96, 3, 0, 16), (3, 64, 2, 32, 32)]
    for a in range(4):
        f1w = [psp.tile([128, 2 * BCORE], F32, tag="ps", name=f"f1w_{a}_{m}")
               for m in range(2)]
        for g, (h0, nh) in enumerate(HG):
            M = nh * 64
            c0 = (g % 2) * BCORE
            ps = f1w[g // 2][:, c0:c0 + BCORE]
            nc.tensor.matmul(ps[0:M, :], W(("rb1", g, a)), W("ones512"),
                             start=True, stop=False, skip_group_check=True)
            nc.tensor.matmul(ps[0:M, :], W(("f1", g, a)), x1t[a][:],
                             start=False, stop=True, skip_group_check=True)
            for hi_ in range(nh):
                hh = h0 + hi_
                # primary region of window a: rows [0:64) = W1 4a..4a+4
                ev_lrelu(x3t[a][0:64, hh * BCORE:(hh + 1) * BCORE],
                         ps[hi_ * 64:hi_ * 64 + 64, :])
        for dst_a, d0, src_a, s0, n in X3_MIR:
            if src_a == a:
                nc.sync.dma_start(x3t[dst_a][d0:d0 + n, :], x3t[a][s0:s0 + n, :])

    # ---- fused2 (a-outer so a3 mirrors can fire early) ----
    # a3 mirrors: (dst_t, d0, src_t, s0, n)
    A3_MIR = [(0, 64, 1, 0, 8), (1, 64, 0, 32, 32), (1, 96, 2, 0, 8),
              (2, 64, 1, 32, 32), (2, 96, 3, 0, 8), (3, 64, 2, 32, 32)]

    def xsl(a, i):
        K = X2_WIN[a][1] * 16 + 1
        return x3t[a][0:K, i * BCORE:(i + 1) * BCORE]

    def f2ev_pair(a, wide, p0, i0, r):
        """One evac for i0 and i0+1: src [64, 2*512] across the wide's banks,
        dst H-cols 5*i0+r and 5*(i0+1)+r via a step-5 slice."""
        H = 5 * i0 + r
        dst = a3t[a][0:64, :].rearrange("p (H b) -> p H b", H=25, b=BCORE)
        ev_lrelu(dst[:, H:H + 6:5, :],
                 wide[p0:p0 + 64, :].rearrange("p (q b) -> p q b", q=2, b=BCORE))

    def f2ev1(a, ps, p0, i, r):
        H = 5 * i + r
        ev_lrelu(a3t[a][0:64, H * BCORE:(H + 1) * BCORE], ps[p0:p0 + 64, :])

    for a in range(4):
        # block-outer so consecutive matmuls share their stationary (LW dedup);
        # i-pairs share a 2-bank wide tile so evacs cover two i at once
        def wides(tag):
            return [psp.tile([128, 2 * BCORE], F32, tag="ps",
                             name=f"f2{tag}_{a}_{m}") for m in range(3)]

        def half(ws, i, rows=128):
            return ws[i // 2][0:rows, (i % 2) * BCORE:(i % 2) * BCORE + BCORE]

        b1 = wides("a")
        for i in range(5):
            nc.tensor.matmul(half(b1, i), WB(("r12", a)), xsl(a, i),
                             start=True, stop=True)
        for i0 in (0, 2):
            for r in (1, 2):
                f2ev_pair(a, b1[i0 // 2], (r - 1) * 64, i0, r)
        f2ev1(a, half(b1, 4), 0, 4, 1)
        f2ev1(a, half(b1, 4), 64, 4, 2)
        b2 = wides("b")
        nc.tensor.matmul(half(b2, 0), WB(("m", "edge", a)), xsl(a, 0),
                         start=True, stop=True, skip_group_check=True)
        for i in range(1, 5):
            nc.tensor.matmul(half(b2, i), WB(("m", "mid", a)), xsl(a, i),
                             start=True, stop=False, skip_group_check=True)
        for i in range(1, 5):
            nc.tensor.matmul(half(b2, i, 64), WB(("r0m1", a)), xsl(a, i - 1),
                             start=False, stop=True, skip_group_check=True)
        for i0 in (0, 2):
            for r in (0, 3):
                f2ev_pair(a, b2[i0 // 2], (r > 0) * 64, i0, r)
        f2ev1(a, half(b2, 4), 0, 4, 0)
        f2ev1(a, half(b2, 4), 64, 4, 3)
        b3 = wides("c")
        for i in range(4):
            nc.tensor.matmul(half(b3, i, 64), WB(("r4", "mid", a)), xsl(a, i),
                             start=True, stop=False, skip_group_check=True)
        nc.tensor.matmul(half(b3, 4, 64), WB(("r4", "edge", a)), xsl(a, 4),
                         start=True, stop=True, skip_group_check=True)
        for i in range(4):
            nc.tensor.matmul(half(b3, i, 64), WB(("r4p1", a)), xsl(a, i + 1),
                             start=False, stop=True, skip_group_check=True)
        for i0 in (0, 2):
            f2ev_pair(a, b3[i0 // 2], 0, i0, 4)
        f2ev1(a, half(b3, 4), 0, 4, 4)
        for dst_t, d0, src_t, s0, n in A3_MIR:
            if src_t == a:
                nc.sync.dma_start(a3t[dst_t][d0:d0 + n, :], a3t[a][s0:s0 + n, :])

    # ---- fused3: merged-MM streams ----
    # bank k = [slot 2k | slot 2k+1], slot = 192 cols (r*96 + c4*16 + wloc).
    # Banks pair into 2-bank wide PSUM tiles; evac covers 4 slots (8 h-rows).
    for c in range(4):
        cb = c * 128
        stg = [stgp.tile([128, 3072], BF16, tag="stg", name=f"stg_{c}_{m}")
               for m in range(7)]
        wides = {}
        for t in range(4):
            Kt = F3_WIN[t][1] * 8 + 1

            def st(i):
                return a3t[t][0:Kt, i * BCORE + cb:i * BCORE + cb + 128]

            for m in range(7):
                wides[(t, m)] = psp.tile([128, 1024], F32, tag="ps",
                                         name=f"f3_{c}_{t}_{m}")

            def bk(k, c0, c1):
                return wides[(t, k // 2)][:, (k % 2) * 512 + c0:(k % 2) * 512 + c1]

            def evac_wide(m):
                nq = 2 if m < 6 else 1
                hq = 4 * nq
                src = wides[(t, m)][:].rearrange("p (q b) -> p q b", q=2, b=512)
                src = src[:, 0:nq, 0:384] if m < 6 else src[:, 0:1, 0:192]
                sv = stg[m][:].rearrange("p (hc w) -> p hc w", hc=48, w=64)
                ev_lrelu(sv[:, 0:(hq if m < 6 else 2) * 6, 16 * t:16 * t + 16], src)
                if t == 3:
                    hr = 8 if m < 6 else 2
                    dview = out_ap[cb:cb + 128, 8 * m:8 * m + hr, :, :]
                    nc.sync.dma_start(dview, stg[m][:, 0:hr * 384])

            cat = ("cat", t)
            for i in range(25):
                k = i // 2
                if i == 0:
                    nc.tensor.matmul(bk(0, 0, 288), st(i), WB(("e0", t)),
                                     start=True, stop=False, skip_group_check=True)
                elif i % 2 == 1:
                    nc.tensor.matmul(bk(k, 96, 384), st(i), WB(cat, 0, 288),
                                     start=False, stop=False, skip_group_check=True)
                    nc.tensor.matmul(bk(k + 1, 0, 96), st(i),
                                     WB(cat, 288, 384), start=True, stop=False,
                                     skip_group_check=True)
                elif i < 24:
                    nc.tensor.matmul(bk(k - 1, 288, 384), st(i),
                                     WB(cat, 0, 96), start=False, stop=True,
                                     skip_group_check=True)
                    nc.tensor.matmul(bk(k, 0, 288), st(i), WB(cat, 96, 384),
                                     start=False, stop=False, skip_group_check=True)
                    if (k - 1) % 2 == 1:
                        evac_wide((k - 1) // 2)
                else:
                    nc.tensor.matmul(bk(11, 288, 384), st(i),
                                     WB(cat, 0, 96), start=False, stop=True,
                                     skip_group_check=True)
                    nc.tensor.matmul(bk(12, 0, 192), st(i), WB(("e24", t)),
                                     start=False, stop=True, skip_group_check=True)
                    evac_wide(5)
                    evac_wide(6)


def kernel(**inputs):
    inputs = {k: np.asarray(v) for k, v in inputs.items()}
    wp, bp, wb = _make_packs(inputs)
    wpack = wp.build()
    bpack = bp.build()
    import ml_dtypes
    wbpack = wb.build().astype(ml_dtypes.bfloat16)
    _build_program.wreg = wp.reg
    _build_program.breg = bp.reg
    _build_program.wbreg = wb.reg
    nc = _build_program(wpack.shape[1], bpack.shape[1], wbpack.shape[1])

    lat = np.ascontiguousarray(inputs["latent"].astype(np.float32))
    ones = np.ones((1, 25 * BCORE), ml_dtypes.bfloat16)
    in_maps = [
        {"latent": lat[i * BCORE:(i + 1) * BCORE], "wpack": wpack,
         "bpack": bpack, "wbpack": wbpack, "ones": ones}
        for i in range(NCORES)
    ]
    res = run_bass_kernel_spmd(nc, in_maps, core_ids=list(range(NCORES)))
    out = np.concatenate(
        [np.asarray(res.results[i]["out"]).astype(np.float32) for i in range(NCORES)],
        axis=0)
    return np.ascontiguousarray(out.transpose(0, 2, 1, 3))  # [B,50,6,64]->[B,6,50,64]


# revision 37
# speedup vs baseline: 1.0778x; 1.0778x over previous
"""Trainium2 Bass kernel for nn_BetaVAEMark7Decoder (v2).

All six layers are matmuls on the TensorEngine; conv pairs are fused on the
host into banded composite blocks (up1*tc1, up2*tc2, up3*tc3). Data-parallel
over batch: 4096 rows split 512 per core.

v2 structural changes vs the 406us baseline:
- fused3 runs as stationary-reuse streams: per (batch-chunk, j-window) the
  a3 activation slice for input row i is loaded once and fires 1-2 merged
  matmuls into a rolling 2-slot-per-bank PSUM ring, relying on PSUM
  has_written semantics (accumulate where written, overwrite where not).
- j-windows (0,9),(4,13),(12,13),(20,12) with 8-wide ownership; window rows
  are permuted so owned rows sit at [0:64) making every fused2 evacuation a
  single full-width [64,512] instruction; halo rows filled by SBUF DMAs.
- biases folded into the matmuls via ones-rows (x3 and a3) so all evacs are
  single-pass lrelu, round-robined across Scalar and Vector engines.
- output staged in bf16 (c4-major, 4 h-rows per tile -> 512B descriptors),
  upcast to f32 on the host.
"""
import numpy as np
from contextlib import ExitStack

import concourse.bass as bass
import concourse.tile as tile
from concourse import bacc, mybir
from concourse.bass_utils import run_bass_kernel_spmd

F32 = mybir.dt.float32
F32R = mybir.dt.float32r
BF16 = mybir.dt.bfloat16
AF = mybir.ActivationFunctionType
OP = mybir.AluOpType

NCORES = 8
BCORE = 512

# fused3 (a3) j-windows over j=W2 in [0,32): (j0, nj); window t owns j in [8t, 8t+8)
F3_WIN = [(0, 9), (4, 13), (12, 13), (20, 12)]
# fused2 input (x3) windows over j=W1 in [0,16): (j0, nj); window a primary j in [4a, 4a+4)
X2_WIN = [(0, 5), (2, 7), (6, 7), (10, 6)]
# fused1 input (x1) windows over wi in [0,8)
X1_WIN = [(0, 3), (1, 4), (3, 4), (5, 3)]
HG = [(0, 2), (2, 2), (4, 1)]


def _x3_row(a, j):
    """Row base (of 16) for x2-col j in x3 window a: primary [0:64), halos after."""
    j0, nj = X2_WIN[a]
    p0 = 4 * a
    if p0 <= j < p0 + 4:
        return (j - p0) * 16
    if j < p0:
        return 64 + (j - j0) * 16
    return 64 + (p0 - j0) * 16 + (j - (p0 + 4)) * 16


def _x3_ones(a):
    return X2_WIN[a][1] * 16


def _a3_row(t, j):
    """Row base (of 8) for W2-col j in a3 window t: owned [0:64), halos after."""
    j0, nj = F3_WIN[t]
    p0 = 8 * t
    if p0 <= j < p0 + 8:
        return (j - p0) * 8
    if j < p0:
        return 64 + (j - j0) * 8
    return 64 + (p0 - j0) * 8 + (j - (p0 + 8)) * 8


def _a3_ones(t):
    return F3_WIN[t][1] * 8


# ---------------- host-side weight factorization ----------------
def _precompute(w):
    P = {}
    w_lin, b_lin = w["w_lin"], w["b_lin"]
    lhs_lin = np.zeros((7, 256), np.float32)
    c_lin = np.zeros(256, np.float32)
    for wi in range(8):
        for ci in range(32):
            lhs_lin[:, wi * 32 + ci] = w_lin[:, ci * 8 + wi]
            c_lin[wi * 32 + ci] = b_lin[ci * 8 + wi]
    P["lhs_lin"], P["c_lin"] = lhs_lin, c_lin

    w_up1, b_up1, w_tc1, b_tc1 = w["w_up1"], w["b_up1"], w["w_tc1"], w["b_tc1"]
    K1 = np.zeros((5, 2, 3, 32, 16), np.float32)
    for hh in range(5):
        for s in range(2):
            for dh in range(3):
                hp = hh + 1 - dh
                if not (0 <= hp < 5):
                    continue
                for dw in range(3):
                    t = s + 1 - dw
                    dj = int(np.floor(t / 2))
                    kw = t - 2 * dj
                    K1[hh, s, dj + 1] += np.einsum("ic,cd->id", w_up1[hp, kw], w_tc1[dh, dw])
    c1 = np.zeros((5, 16, 16), np.float32)
    for hh in range(5):
        for ww in range(16):
            acc = b_tc1.copy()
            for dh in range(3):
                if not (0 <= hh + 1 - dh < 5):
                    continue
                for dw in range(3):
                    if not (0 <= ww + 1 - dw < 16):
                        continue
                    acc = acc + b_up1 @ w_tc1[dh, dw]
            c1[hh, ww] = acc
    P["K1"], P["c1"] = K1, c1

    w_up2, b_up2, w_tc2, b_tc2 = w["w_up2"], w["b_up2"], w["w_tc2"], w["b_tc2"]
    K2 = np.zeros((5, 2, 3, 3, 16, 8), np.float32)
    for r in range(5):
        for s in range(2):
            for dh in range(3):
                u = r + 1 - dh
                di = int(np.floor(u / 5))
                kh = u - 5 * di
                for dw in range(3):
                    t = s + 1 - dw
                    dj = int(np.floor(t / 2))
                    kw = t - 2 * dj
                    K2[r, s, di + 1, dj + 1] += np.einsum("ic,cd->id", w_up2[kh, kw], w_tc2[dh, dw])
    P["K2"] = K2
    P["BB2"] = np.einsum("c,hwcd->hwd", b_up2, w_tc2)
    P["b_tc2"] = b_tc2

    w_up3, b_up3, w_tc3, b_tc3 = w["w_up3"], w["b_up3"], w["w_tc3"], w["b_tc3"]
    K3 = np.zeros((2, 2, 3, 3, 8, 6), np.float32)
    for r in range(2):
        for s in range(2):
            for dh in range(3):
                u = r + 1 - dh
                di = int(np.floor(u / 2))
                kh = u - 2 * di
                for dw in range(3):
                    t = s + 1 - dw
                    dj = int(np.floor(t / 2))
                    kw = t - 2 * dj
                    K3[r, s, di + 1, dj + 1] += np.einsum("ic,cd->id", w_up3[kh, kw], w_tc3[dh, dw])
    P["K3"] = K3
    P["BB3"] = np.einsum("c,hwcd->hwd", b_up3, w_tc3)
    P["b_tc3"] = b_tc3
    return P


def _fused1_blocks(P):
    K1 = P["K1"]
    blocks, biases = {}, {}
    for g, (h0, nh) in enumerate(HG):
        for a in range(4):
            wi0, nwi = X1_WIN[a]
            M = nh * 4 * 16
            B = np.zeros((nwi * 32, M), np.float32)
            bias = np.zeros(M, np.float32)
            for hi in range(nh):
                hh = h0 + hi
                for wl in range(4):
                    ww = 4 * a + wl
                    j, s = ww // 2, ww % 2
                    for c2 in range(16):
                        col = hi * 64 + wl * 16 + c2
                        bias[col] = P["c1"][hh, ww, c2]
                        for wi_l in range(nwi):
                            dj = (wi0 + wi_l) - j
                            if -1 <= dj <= 1:
                                B[wi_l * 32:(wi_l + 1) * 32, col] = K1[hh, s, dj + 1, :, c2]
            blocks[(g, a)] = B
            biases[(g, a)] = bias
    return blocks, biases


def _f2_col_bias(P, Hh, Ww, c3):
    acc = P["b_tc2"][c3]
    for dh in range(3):
        if not (0 <= Hh + 1 - dh < 25):
            continue
        for dw in range(3):
            if not (0 <= Ww + 1 - dw < 32):
                continue
            acc += P["BB2"][dh, dw, c3]
    return acc


def _fused2_blocks(P):
    """Blocks with x3 row permutation and bias rows at the ones-row position."""
    K2 = P["K2"]
    blocks = {}
    for a in range(4):
        j0, nj = X2_WIN[a]
        K = nj * 16 + 1
        ones = _x3_ones(a)

        def fill(B, colbase, r, di, bias_i=None):
            for wl in range(8):
                Ww = 8 * a + wl
                j, s = Ww // 2, Ww % 2
                for c3 in range(8):
                    col = colbase + wl * 8 + c3
                    for j2 in range(j0, j0 + nj):
                        dj = j2 - j
                        if -1 <= dj <= 1:
                            rb = _x3_row(a, j2)
                            B[rb:rb + 16, col] = K2[r, s, di + 1, dj + 1, :, c3]
                    if bias_i is not None:
                        B[ones, col] = _f2_col_bias(P, 5 * bias_i + r, Ww, c3)

        B = np.zeros((K, 128), np.float32)
        fill(B, 0, 1, 0, bias_i=1)
        fill(B, 64, 2, 0, bias_i=1)
        blocks[("r12", a)] = B
        for tag, bi in (("mid", 2), ("edge", 0)):
            B = np.zeros((K, 128), np.float32)
            fill(B, 0, 0, 0, bias_i=bi)
            fill(B, 64, 3, 0, bias_i=1)
            blocks[("m", tag, a)] = B
        B = np.zeros((K, 64), np.float32)
        fill(B, 0, 0, -1)
        blocks[("r0m1", a)] = B
        for tag, bi in (("mid", 2), ("edge", 4)):
            B = np.zeros((K, 64), np.float32)
            fill(B, 0, 4, 0, bias_i=bi)
            blocks[("r4", tag, a)] = B
        B = np.zeros((K, 64), np.float32)
        fill(B, 0, 4, 1)
        blocks[("r4p1", a)] = B
    return blocks


def _fused3_blocks(P):
    """Per t: cat [K,384] = [W(+1)r1 | W(0) | W(-1)r0], e0 [K,288], e24 [K,192].
    Slot col order r*96 + c4*16 + (jc-8t)*2 + s; a3 row permutation applied."""
    K3, BB3, b_tc3 = P["K3"], P["BB3"], P["b_tc3"]
    blocks = {}
    for t in range(4):
        j0, nj = F3_WIN[t]
        K = nj * 8 + 1
        ones = _a3_ones(t)

        def w_block(di, rsel, iclass=None):
            B = np.zeros((K, len(rsel) * 96), np.float32)
            for ri, r in enumerate(rsel):
                for c4 in range(6):
                    for jc in range(8 * t, 8 * t + 8):
                        for s in range(2):
                            col = ri * 96 + c4 * 16 + (jc - 8 * t) * 2 + s
                            for j2 in range(j0, j0 + nj):
                                dj = j2 - jc
                                if -1 <= dj <= 1:
                                    rb = _a3_row(t, j2)
                                    B[rb:rb + 8, col] = K3[r, s, di + 1, dj + 1, :, c4]
                            if iclass is not None and di == 0:
                                acc = b_tc3[c4]
                                for dh in range(3):
                                    u = r + 1 - dh
                                    di_ = int(np.floor(u / 2))
                                    ok = (iclass == 0) or (iclass == 1 and di_ >= 0) \
                                        or (iclass == 2 and di_ <= 0)
                                    if not ok:
                                        continue
                                    for dw in range(3):
                                        tt = s + 1 - dw
                                        dj_ = int(np.floor(tt / 2))
                                        if 0 <= jc + dj_ < 32:
                                            acc += BB3[dh, dw, c4]
                                B[ones, col] = acc
            return B

        w1r1 = w_block(1, [1])
        wm1r0 = w_block(-1, [0])
        blocks[("cat", t)] = np.concatenate([w1r1, w_block(0, [0, 1], 0), wm1r0], axis=1)
        blocks[("e0", t)] = np.concatenate([w_block(0, [0, 1], 1), wm1r0], axis=1)
        blocks[("e24", t)] = w_block(0, [0, 1], 2)
    return blocks


class _Pack:
    def __init__(self):
        self.cols = 0
        self.reg = {}
        self.items = []

    def add(self, key, arr):
        K, M = arr.shape
        self.reg[key] = (self.cols, K, M)
        self.items.append(arr)
        self.cols += M

    def build(self):
        out = np.zeros((128, self.cols), np.float32)
        c = 0
        for arr in self.items:
            K, M = arr.shape
            out[:K, c:c + M] = arr
            c += M
        return out


def _make_packs(inputs):
    P = _precompute(inputs)
    f1b, f1bias = _fused1_blocks(P)
    f2b = _fused2_blocks(P)
    f3b = _fused3_blocks(P)

    # order: lin + bias rows first so the first two 512-col chunks unblock lin
    wp = _Pack()
    wp.add("lin0", P["lhs_lin"][:, 0:128])
    wp.add("lin1", P["lhs_lin"][:, 128:256])
    # row-form biases + a ones row: biases enter PSUM via a K=1 matmul
    wp.add("ones512", np.ones((1, BCORE), np.float32))
    wp.add("rblin0", P["c_lin"][0:128].reshape(1, -1))
    wp.add("rblin1", P["c_lin"][128:256].reshape(1, -1))
    for g in range(3):
        for a in range(4):
            wp.add(("rb1", g, a), f1bias[(g, a)].reshape(1, -1))
    for g in range(3):
        for a in range(4):
            wp.add(("f1", g, a), f1b[(g, a)])

    wb = _Pack()
    for a in range(4):
        for key in [("r12", a), ("m", "mid", a), ("m", "edge", a), ("r0m1", a),
                    ("r4", "mid", a), ("r4", "edge", a), ("r4p1", a)]:
            wb.add(key, f2b[key])
    for t in range(4):
        for key in [("cat", t), ("e0", t), ("e24", t)]:
            wb.add(key, f3b[key])

    bp = _Pack()
    bp.add("blin0", P["c_lin"][0:128].reshape(-1, 1))
    bp.add("blin1", P["c_lin"][128:256].reshape(-1, 1))
    for g in range(3):
        for a in range(4):
            bp.add(("b1", g, a), f1bias[(g, a)].reshape(-1, 1))
    return wp, bp, wb


# ---------------- device program ----------------
_PROG = {}


def _lim(s):
    if s == 0:
        return 128
    if s == 64:
        return 64
    return 32


def _pieces(p0, d0, n):
    assert p0 % 32 == 0 and d0 % 32 == 0, (p0, d0, n)
    out = []
    off = 0
    while off < n:
        s1, s2 = (p0 + off) % 128, (d0 + off) % 128
        c = min(n - off, _lim(s1), _lim(s2))
        out.append((off, c))
        off += c
    return out


def _build_program(wcols, bcols, wbcols):
    key = (wcols, bcols, wbcols)
    if key in _PROG:
        return _PROG[key]
    nc = bacc.Bacc("TRN2", target_bir_lowering=False, debug=False, num_devices=NCORES)
    lat_ap = nc.dram_tensor("latent", [BCORE, 7], F32, kind="ExternalInput").ap()
    wp_ap = nc.dram_tensor("wpack", [128, wcols], F32, kind="ExternalInput").ap()
    bp_ap = nc.dram_tensor("bpack", [128, bcols], F32, kind="ExternalInput").ap()
    wb_ap = nc.dram_tensor("wbpack", [128, wbcols], BF16, kind="ExternalInput").ap()
    # h-major output (host transposes to NCHW): fully contiguous stg DMA
    out_ap = nc.dram_tensor("out", [BCORE, 50, 6, 64], BF16, kind="ExternalOutput").ap()
    ones_ap = nc.dram_tensor("ones", [1, 25 * BCORE], BF16, kind="ExternalInput").ap()
    with tile.TileContext(nc) as tc:
        with ExitStack() as ctx:
            _emit(ctx, tc, nc, lat_ap, wp_ap, bp_ap, wb_ap, out_ap, ones_ap,
                  _build_program.wreg, _build_program.breg, _build_program.wbreg)
    _dedup_ldweights(nc)
    nc.compile()
    _PROG[key] = nc
    return nc


def _dedup_ldweights(nc):
    """Drop InstLdweights whose stationary AP matches the previous load on the
    PE queue (the PE array keeps the stationary across matmuls)."""
    from concourse import mybir
    removed = 0
    for fn in nc.m.functions:
        for blk in fn.blocks:
            insts = list(blk.instructions)
            keep = []
            prev_sig = None
            for ins in insts:
                tn = type(ins).__name__
                if tn == "InstLdweights":
                    sig = (str(ins.ins[0]), str(getattr(ins, "perf_mode", None)),
                           str(getattr(ins, "is_transpose", None)))
                    if sig == prev_sig and not ins.has_wait() and not ins.has_update():
                        removed += 1
                        continue
                    prev_sig = sig
                elif tn == "InstMatmult":
                    pass  # streaming doesn't clobber the loaded stationary
                elif getattr(ins, "engine", None) == mybir.EngineType.PE \
                        and tn not in ("InstEventSemaphore",):
                    prev_sig = None
                keep.append(ins)
            if len(keep) != len(insts):
                blk.instructions = keep
    return removed


def _emit(ctx, tc, nc, lat_ap, wp_ap, bp_ap, wb_ap, out_ap, ones_ap, wreg, breg, wbreg):
    wcols = wp_ap.shape[1]
    bcols = bp_ap.shape[1]
    wbcols = wb_ap.shape[1]

    consts = ctx.enter_context(tc.tile_pool(name="consts", bufs=1))
    bounce = ctx.enter_context(tc.tile_pool(name="bounce", bufs=2))
    x1p = ctx.enter_context(tc.tile_pool(name="x1", bufs=1))
    x3p = ctx.enter_context(tc.tile_pool(name="x3", bufs=1))
    a3p = ctx.enter_context(tc.tile_pool(name="a3", bufs=1))
    stgp = ctx.enter_context(tc.tile_pool(name="stg", bufs=14))
    psp = ctx.enter_context(tc.tile_pool(name="ps", bufs=8, space="PSUM"))

    # ---- constants (lin dependencies first, big fused-weight pack last) ----
    lat_f = consts.tile([7, BCORE], F32)
    nc.sync.dma_start(lat_f[:], lat_ap[:].rearrange("b d -> d b"))
    lat_r = consts.tile([7, BCORE], F32R)
    nc.vector.tensor_copy(lat_r[:], lat_f[:])
    wp_r = consts.tile([128, wcols], F32R)
    for c0 in range(0, wcols, 512):
        n = min(512, wcols - c0)
        bt = bounce.tile([128, 512], F32, tag="bounce", name=f"bw{c0}")
        nc.sync.dma_start(bt[:, :n], wp_ap[:, c0:c0 + n])
        nc.vector.tensor_copy(wp_r[:, c0:c0 + n], bt[:, :n])
    wbt = consts.tile([128, wbcols], BF16)
    nc.sync.dma_start(wbt[:], wb_ap[:])
    bpt = consts.tile([128, bcols], F32)
    nc.sync.dma_start(bpt[:], bp_ap[:])

    def W(key):
        o, K, M = wreg[key]
        return wp_r[:K, o:o + M]

    def WB(key, c0=None, c1=None):
        o, K, M = wbreg[key]
        if c0 is None:
            return wbt[:K, o:o + M]
        return wbt[:K, o + c0:o + c1]

    def BV(key, p0, n):
        o, K, M = breg[key]
        return bpt[p0:p0 + n, o:o + 1]

    # evac engines: ACT does lrelu via activation, DVE via scalar_tensor_tensor
    ev_ctr = [0]

    def ev_lrelu(dst, src):
        # DVE cannot read two PSUM operands in one instruction (and Pool
        # rejects stt entirely): DVE path = PSUM->SBUF copy + in-place lrelu.
        # ACT single-pass is cheaper: 3 of every 5.
        if ev_ctr[0] % 5 in (0, 2, 4):
            nc.scalar.activation(dst, src, AF.Lrelu, bias=0.0, scale=1.0, alpha=0.01)
        else:
            nc.vector.tensor_copy(dst, src)
            nc.vector.scalar_tensor_tensor(dst, dst, 0.01, dst, op0=OP.mult, op1=OP.max)
        ev_ctr[0] += 1

    # ---- x3 / a3 tiles + ones rows ----
    x3t = [x3p.tile([X2_WIN[a][1] * 16 + 1, 5 * BCORE], BF16, tag=f"x3_{a}",
                    name=f"x3_{a}") for a in range(4)]
    a3t = [a3p.tile([F3_WIN[t][1] * 8 + 1, 25 * BCORE], BF16, tag=f"a3_{t}",
                    name=f"a3_{t}") for t in range(4)]
    # ones rows via DMA from a DRAM constant (gpsimd memset is ~10us per row)
    for a in range(4):
        o = _x3_ones(a)
        nc.sync.dma_start(x3t[a][o:o + 1, :], ones_ap[0:1, 0:5 * BCORE])
    for t in range(4):
        o = _a3_ones(t)
        nc.sync.dma_start(a3t[t][o:o + 1, :], ones_ap[0:1, :])

    # ---- lin (bias preloaded into PSUM via K=1 matmul against a ones row) ----
    psA = psp.tile([128, BCORE], F32, tag="ps", name="linA")
    nc.tensor.matmul(psA[:], W("rblin0"), W("ones512"), start=True, stop=False,
                     skip_group_check=True)
    nc.tensor.matmul(psA[:], W("lin0"), lat_r[:], start=False, stop=True,
                     skip_group_check=True)
    psB = psp.tile([128, BCORE], F32, tag="ps", name="linB")
    nc.tensor.matmul(psB[:], W("rblin1"), W("ones512"), start=True, stop=False,
                     skip_group_check=True)
    nc.tensor.matmul(psB[:], W("lin1"), lat_r[:], start=False, stop=True,
                     skip_group_check=True)

    x1t = [x1p.tile([X1_WIN[a][1] * 32, BCORE], F32R, tag=f"x1_{a}", name=f"x1_{a}")
           for a in range(4)]
    for a in range(4):
        wi0, nwi = X1_WIN[a]
        for ps, base in ((psA, 0), (psB, 4)):
            lo = max(wi0, base)
            hi = min(wi0 + nwi, base + 4)
            if lo >= hi:
                continue
            d0 = (lo - wi0) * 32
            p0 = (lo - base) * 32
            n = (hi - lo) * 32
            for off, cnt in _pieces(p0, d0, n):
                ev_lrelu(x1t[a][d0 + off:d0 + off + cnt, :],
                         ps[p0 + off:p0 + off + cnt, :])

    # ---- fused1 (a-outer so x3 mirrors can fire early) ----
    # x3 mirrors: (dst_a, d0, src_a, s0, n)
    X3_MIR = [(0, 64, 1, 0, 16), (1, 64, 0, 32, 32), (1, 96, 2, 0, 16),
              (2, 64, 1, 32, 32), (2, 96, 3, 0, 16), (3, 64, 2, 32, 32)]
    for a in range(4):
        for g, (h0, nh) in enumerate(HG):
            M = nh * 64
            ps = psp.tile([128, BCORE], F32, tag="ps", name=f"f1_{g}_{a}")
            nc.tensor.matmul(ps[0:M, :], W(("rb1", g, a)), W("ones512"),
                             start=True, stop=False, skip_group_check=True)
            nc.tensor.matmul(ps[0:M, :], W(("f1", g, a)), x1t[a][:],
                             start=False, stop=True, skip_group_check=True)
            for hi_ in range(nh):
                hh = h0 + hi_
                # primary region of window a: rows [0:64) = W1 4a..4a+4
                ev_lrelu(x3t[a][0:64, hh * BCORE:(hh + 1) * BCORE],
                         ps[hi_ * 64:hi_ * 64 + 64, :])
        for dst_a, d0, src_a, s0, n in X3_MIR:
            if src_a == a:
                nc.sync.dma_start(x3t[dst_a][d0:d0 + n, :], x3t[a][s0:s0 + n, :])

    # ---- fused2 (a-outer so a3 mirrors can fire early) ----
    # a3 mirrors: (dst_t, d0, src_t, s0, n)
    A3_MIR = [(0, 64, 1, 0, 8), (1, 64, 0, 32, 32), (1, 96, 2, 0, 8),
              (2, 64, 1, 32, 32), (2, 96, 3, 0, 8), (3, 64, 2, 32, 32)]

    def xsl(a, i):
        K = X2_WIN[a][1] * 16 + 1
        return x3t[a][0:K, i * BCORE:(i + 1) * BCORE]

    def f2ev1(a, ps, p0, i, r):
        H = 5 * i + r
        ev_lrelu(a3t[a][0:64, H * BCORE:(H + 1) * BCORE], ps[p0:p0 + 64, :])

    for a in range(4):
        # block-outer so consecutive matmuls share their stationary (LW dedup)
        b1 = [psp.tile([128, BCORE], F32, tag="ps", name=f"f2a_{a}_{i}")
              for i in range(5)]
        for i in range(5):
            nc.tensor.matmul(b1[i][:], WB(("r12", a)), xsl(a, i),
                             start=True, stop=True)
            f2ev1(a, b1[i], 0, i, 1)
            f2ev1(a, b1[i], 64, i, 2)
        b2 = [psp.tile([128, BCORE], F32, tag="ps", name=f"f2b_{a}_{i}")
              for i in range(5)]
        nc.tensor.matmul(b2[0][:], WB(("m", "edge", a)), xsl(a, 0),
                         start=True, stop=True, skip_group_check=True)
        f2ev1(a, b2[0], 0, 0, 0)
        f2ev1(a, b2[0], 64, 0, 3)
        for i in range(1, 5):
            nc.tensor.matmul(b2[i][:], WB(("m", "mid", a)), xsl(a, i),
                             start=True, stop=False, skip_group_check=True)
        for i in range(1, 5):
            nc.tensor.matmul(b2[i][0:64, :], WB(("r0m1", a)), xsl(a, i - 1),
                             start=False, stop=True, skip_group_check=True)
            f2ev1(a, b2[i], 0, i, 0)
            f2ev1(a, b2[i], 64, i, 3)
        b3 = [psp.tile([128, BCORE], F32, tag="ps", name=f"f2c_{a}_{i}")
              for i in range(5)]
        for i in range(4):
            nc.tensor.matmul(b3[i][0:64, :], WB(("r4", "mid", a)), xsl(a, i),
                             start=True, stop=False, skip_group_check=True)
        nc.tensor.matmul(b3[4][0:64, :], WB(("r4", "edge", a)), xsl(a, 4),
                         start=True, stop=True, skip_group_check=True)
        f2ev1(a, b3[4], 0, 4, 4)
        for i in range(4):
            nc.tensor.matmul(b3[i][0:64, :], WB(("r4p1", a)), xsl(a, i + 1),
                             start=False, stop=True, skip_group_check=True)
            f2ev1(a, b3[i], 0, i, 4)
        for dst_t, d0, src_t, s0, n in A3_MIR:
            if src_t == a:
                nc.sync.dma_start(a3t[dst_t][d0:d0 + n, :], a3t[a][s0:s0 + n, :])

    # ---- fused3: merged-MM streams ----
    # bank k = [slot 2k | slot 2k+1], slot = 192 cols (r*96 + c4*16 + wloc).
    for c in range(4):
        cb = c * 128
        stg = [stgp.tile([128, 1536], BF16, tag="stg", name=f"stg_{c}_{k}")
               for k in range(13)]
        banks = {}
        for t in range(4):
            Kt = F3_WIN[t][1] * 8 + 1

            def st(i):
                return a3t[t][0:Kt, i * BCORE + cb:i * BCORE + cb + 128]

            for k in range(13):
                banks[(t, k)] = psp.tile([128, 512], F32, tag="ps",
                                         name=f"f3_{c}_{t}_{k}")

            def evac(k):
                ps = banks[(t, k)]
                n = 384 if k < 12 else 192
                hq = 4 if k < 12 else 2
                # stg cols (h, c4, w) h-major: PSUM order (slot,r,c4,w) is
                # h-major too, so dst collapses to 3D (p, h*c4 chunk, w)
                sv = stg[k][:].rearrange("p (hc w) -> p hc w", hc=24, w=64)
                ev_lrelu(sv[:, 0:hq * 6, 16 * t:16 * t + 16], ps[:, 0:n])
                if t == 3:
                    dview = out_ap[cb:cb + 128, 4 * k:4 * k + hq, :, :]
                    nc.sync.dma_start(dview, stg[k][:, 0:hq * 384])

            cat = ("cat", t)
            for i in range(25):
                k = i // 2
                if i == 0:
                    nc.tensor.matmul(banks[(t, 0)][:, 0:288], st(i), WB(("e0", t)),
                                     start=True, stop=False, skip_group_check=True)
                elif i % 2 == 1:
                    nc.tensor.matmul(banks[(t, k)][:, 96:384], st(i), WB(cat, 0, 288),
                                     start=False, stop=False, skip_group_check=True)
                    nc.tensor.matmul(banks[(t, k + 1)][:, 0:96], st(i),
                                     WB(cat, 288, 384), start=True, stop=False,
                                     skip_group_check=True)
                elif i < 24:
                    nc.tensor.matmul(banks[(t, k - 1)][:, 288:384], st(i),
                                     WB(cat, 0, 96), start=False, stop=True,
                                     skip_group_check=True)
                    nc.tensor.matmul(banks[(t, k)][:, 0:288], st(i), WB(cat, 96, 384),
                                     start=False, stop=False, skip_group_check=True)
                    evac(k - 1)
                else:
                    nc.tensor.matmul(banks[(t, 11)][:, 288:384], st(i),
                                     WB(cat, 0, 96), start=False, stop=True,
                                     skip_group_check=True)
                    nc.tensor.matmul(banks[(t, 12)][:, 0:192], st(i), WB(("e24", t)),
                                     start=False, stop=True, skip_group_check=True)
                    evac(11)
                    evac(12)


def kernel(**inputs):
    inputs = {k: np.asarray(v) for k, v in inputs.items()}
    wp, bp, wb = _make_packs(inputs)
    wpack = wp.build()
    bpack = bp.build()
    import ml_dtypes
    wbpack = wb.build().astype(ml_dtypes.bfloat16)
    _build_program.wreg = wp.reg
    _build_program.breg = bp.reg
    _build_program.wbreg = wb.reg
    nc = _build_program(wpack.shape[1], bpack.shape[1], wbpack.shape[1])

    lat = np.ascontiguousarray(inputs["latent"].astype(np.float32))
    ones = np.ones((1, 25 * BCORE), ml_dtypes.bfloat16)
    in_maps = [
        {"latent": lat[i * BCORE:(i + 1) * BCORE], "wpack": wpack,
         "bpack": bpack, "wbpack": wbpack, "ones": ones}
        for i in range(NCORES)
    ]
    res = run_bass_kernel_spmd(nc, in_maps, core_ids=list(range(NCORES)))
    out = np.concatenate(
        [np.asarray(res.results[i]["out"]).astype(np.float32) for i in range(NCORES)],
        axis=0)
    return np.ascontiguousarray(out.transpose(0, 2, 1, 3))  # [B,50,6,64]->[B,6,50,64]


# revision 41
# speedup vs baseline: 1.0846x; 1.0064x over previous
"""Trainium2 Bass kernel for nn_BetaVAEMark7Decoder (v2).

All six layers are matmuls on the TensorEngine; conv pairs are fused on the
host into banded composite blocks (up1*tc1, up2*tc2, up3*tc3). Data-parallel
over batch: 4096 rows split 512 per core.

v2 structural changes vs the 406us baseline:
- fused3 runs as stationary-reuse streams: per (batch-chunk, j-window) the
  a3 activation slice for input row i is loaded once and fires 1-2 merged
  matmuls into a rolling 2-slot-per-bank PSUM ring, relying on PSUM
  has_written semantics (accumulate where written, overwrite where not).
- j-windows (0,9),(4,13),(12,13),(20,12) with 8-wide ownership; window rows
  are permuted so owned rows sit at [0:64) making every fused2 evacuation a
  single full-width [64,512] instruction; halo rows filled by SBUF DMAs.
- biases folded into the matmuls via ones-rows (x3 and a3) so all evacs are
  single-pass lrelu, round-robined across Scalar and Vector engines.
- output staged in bf16 (c4-major, 4 h-rows per tile -> 512B descriptors),
  upcast to f32 on the host.
"""
import numpy as np
from contextlib import ExitStack

import concourse.bass as bass
import concourse.tile as tile
from concourse import bacc, mybir
from concourse.bass_utils import run_bass_kernel_spmd

F32 = mybir.dt.float32
F32R = mybir.dt.float32r
BF16 = mybir.dt.bfloat16
AF = mybir.ActivationFunctionType
OP = mybir.AluOpType

NCORES = 8
BCORE = 512

# fused3 (a3) j-windows over j=W2 in [0,32): (j0, nj); window t owns j in [8t, 8t+8)
F3_WIN = [(0, 9), (4, 13), (12, 13), (20, 12)]
# fused2 input (x3) windows over j=W1 in [0,16): (j0, nj); window a primary j in [4a, 4a+4)
X2_WIN = [(0, 5), (2, 7), (6, 7), (10, 6)]
# fused1 input (x1) windows over wi in [0,8)
X1_WIN = [(0, 3), (1, 4), (3, 4), (5, 3)]
HG = [(0, 2), (2, 2), (4, 1)]


def _x3_row(a, j):
    """Row base (of 16) for x2-col j in x3 window a: primary [0:64), halos after."""
    j0, nj = X2_WIN[a]
    p0 = 4 * a
    if p0 <= j < p0 + 4:
        return (j - p0) * 16
    if j < p0:
        return 64 + (j - j0) * 16
    return 64 + (p0 - j0) * 16 + (j - (p0 + 4)) * 16


def _x3_ones(a):
    return X2_WIN[a][1] * 16


def _a3_row(t, j):
    """Row base (of 8) for W2-col j in a3 window t: owned [0:64), halos after."""
    j0, nj = F3_WIN[t]
    p0 = 8 * t
    if p0 <= j < p0 + 8:
        return (j - p0) * 8
    if j < p0:
        return 64 + (j - j0) * 8
    return 64 + (p0 - j0) * 8 + (j - (p0 + 8)) * 8


def _a3_ones(t):
    return F3_WIN[t][1] * 8


# ---------------- host-side weight factorization ----------------
def _precompute(w):
    P = {}
    w_lin, b_lin = w["w_lin"], w["b_lin"]
    lhs_lin = np.zeros((7, 256), np.float32)
    c_lin = np.zeros(256, np.float32)
    for wi in range(8):
        for ci in range(32):
            lhs_lin[:, wi * 32 + ci] = w_lin[:, ci * 8 + wi]
            c_lin[wi * 32 + ci] = b_lin[ci * 8 + wi]
    P["lhs_lin"], P["c_lin"] = lhs_lin, c_lin

    w_up1, b_up1, w_tc1, b_tc1 = w["w_up1"], w["b_up1"], w["w_tc1"], w["b_tc1"]
    K1 = np.zeros((5, 2, 3, 32, 16), np.float32)
    for hh in range(5):
        for s in range(2):
            for dh in range(3):
                hp = hh + 1 - dh
                if not (0 <= hp < 5):
                    continue
                for dw in range(3):
                    t = s + 1 - dw
                    dj = int(np.floor(t / 2))
                    kw = t - 2 * dj
                    K1[hh, s, dj + 1] += np.einsum("ic,cd->id", w_up1[hp, kw], w_tc1[dh, dw])
    c1 = np.zeros((5, 16, 16), np.float32)
    for hh in range(5):
        for ww in range(16):
            acc = b_tc1.copy()
            for dh in range(3):
                if not (0 <= hh + 1 - dh < 5):
                    continue
                for dw in range(3):
                    if not (0 <= ww + 1 - dw < 16):
                        continue
                    acc = acc + b_up1 @ w_tc1[dh, dw]
            c1[hh, ww] = acc
    P["K1"], P["c1"] = K1, c1

    w_up2, b_up2, w_tc2, b_tc2 = w["w_up2"], w["b_up2"], w["w_tc2"], w["b_tc2"]
    K2 = np.zeros((5, 2, 3, 3, 16, 8), np.float32)
    for r in range(5):
        for s in range(2):
            for dh in range(3):
                u = r + 1 - dh
                di = int(np.floor(u / 5))
                kh = u - 5 * di
                for dw in range(3):
                    t = s + 1 - dw
                    dj = int(np.floor(t / 2))
                    kw = t - 2 * dj
                    K2[r, s, di + 1, dj + 1] += np.einsum("ic,cd->id", w_up2[kh, kw], w_tc2[dh, dw])
    P["K2"] = K2
    P["BB2"] = np.einsum("c,hwcd->hwd", b_up2, w_tc2)
    P["b_tc2"] = b_tc2

    w_up3, b_up3, w_tc3, b_tc3 = w["w_up3"], w["b_up3"], w["w_tc3"], w["b_tc3"]
    K3 = np.zeros((2, 2, 3, 3, 8, 6), np.float32)
    for r in range(2):
        for s in range(2):
            for dh in range(3):
                u = r + 1 - dh
                di = int(np.floor(u / 2))
                kh = u - 2 * di
                for dw in range(3):
                    t = s + 1 - dw
                    dj = int(np.floor(t / 2))
                    kw = t - 2 * dj
                    K3[r, s, di + 1, dj + 1] += np.einsum("ic,cd->id", w_up3[kh, kw], w_tc3[dh, dw])
    P["K3"] = K3
    P["BB3"] = np.einsum("c,hwcd->hwd", b_up3, w_tc3)
    P["b_tc3"] = b_tc3
    return P


def _fused1_blocks(P):
    K1 = P["K1"]
    blocks, biases = {}, {}
    for g, (h0, nh) in enumerate(HG):
        for a in range(4):
            wi0, nwi = X1_WIN[a]
            M = nh * 4 * 16
            B = np.zeros((nwi * 32, M), np.float32)
            bias = np.zeros(M, np.float32)
            for hi in range(nh):
                hh = h0 + hi
                for wl in range(4):
                    ww = 4 * a + wl
                    j, s = ww // 2, ww % 2
                    for c2 in range(16):
                        col = hi * 64 + wl * 16 + c2
                        bias[col] = P["c1"][hh, ww, c2]
                        for wi_l in range(nwi):
                            dj = (wi0 + wi_l) - j
                            if -1 <= dj <= 1:
                                B[wi_l * 32:(wi_l + 1) * 32, col] = K1[hh, s, dj + 1, :, c2]
            blocks[(g, a)] = B
            biases[(g, a)] = bias
    return blocks, biases


def _f2_col_bias(P, Hh, Ww, c3):
    acc = P["b_tc2"][c3]
    for dh in range(3):
        if not (0 <= Hh + 1 - dh < 25):
            continue
        for dw in range(3):
            if not (0 <= Ww + 1 - dw < 32):
                continue
            acc += P["BB2"][dh, dw, c3]
    return acc


def _fused2_blocks(P):
    """Blocks with x3 row permutation and bias rows at the ones-row position."""
    K2 = P["K2"]
    blocks = {}
    for a in range(4):
        j0, nj = X2_WIN[a]
        K = nj * 16 + 1
        ones = _x3_ones(a)

        def fill(B, colbase, r, di, bias_i=None):
            for wl in range(8):
                Ww = 8 * a + wl
                j, s = Ww // 2, Ww % 2
                for c3 in range(8):
                    col = colbase + wl * 8 + c3
                    for j2 in range(j0, j0 + nj):
                        dj = j2 - j
                        if -1 <= dj <= 1:
                            rb = _x3_row(a, j2)
                            B[rb:rb + 16, col] = K2[r, s, di + 1, dj + 1, :, c3]
                    if bias_i is not None:
                        B[ones, col] = _f2_col_bias(P, 5 * bias_i + r, Ww, c3)

        B = np.zeros((K, 128), np.float32)
        fill(B, 0, 1, 0, bias_i=1)
        fill(B, 64, 2, 0, bias_i=1)
        blocks[("r12", a)] = B
        for tag, bi in (("mid", 2), ("edge", 0)):
            B = np.zeros((K, 128), np.float32)
            fill(B, 0, 0, 0, bias_i=bi)
            fill(B, 64, 3, 0, bias_i=1)
            blocks[("m", tag, a)] = B
        B = np.zeros((K, 64), np.float32)
        fill(B, 0, 0, -1)
        blocks[("r0m1", a)] = B
        for tag, bi in (("mid", 2), ("edge", 4)):
            B = np.zeros((K, 64), np.float32)
            fill(B, 0, 4, 0, bias_i=bi)
            blocks[("r4", tag, a)] = B
        B = np.zeros((K, 64), np.float32)
        fill(B, 0, 4, 1)
        blocks[("r4p1", a)] = B
    return blocks


def _fused3_blocks(P):
    """Per t: cat [K,384] = [W(+1)r1 | W(0) | W(-1)r0], e0 [K,288], e24 [K,192].
    Slot col order r*96 + c4*16 + (jc-8t)*2 + s; a3 row permutation applied."""
    K3, BB3, b_tc3 = P["K3"], P["BB3"], P["b_tc3"]
    blocks = {}
    for t in range(4):
        j0, nj = F3_WIN[t]
        K = nj * 8 + 1
        ones = _a3_ones(t)

        def w_block(di, rsel, iclass=None):
            B = np.zeros((K, len(rsel) * 96), np.float32)
            for ri, r in enumerate(rsel):
                for c4 in range(6):
                    for jc in range(8 * t, 8 * t + 8):
                        for s in range(2):
                            col = ri * 96 + c4 * 16 + (jc - 8 * t) * 2 + s
                            for j2 in range(j0, j0 + nj):
                                dj = j2 - jc
                                if -1 <= dj <= 1:
                                    rb = _a3_row(t, j2)
                                    B[rb:rb + 8, col] = K3[r, s, di + 1, dj + 1, :, c4]
                            if iclass is not None and di == 0:
                                acc = b_tc3[c4]
                                for dh in range(3):
                                    u = r + 1 - dh
                                    di_ = int(np.floor(u / 2))
                                    ok = (iclass == 0) or (iclass == 1 and di_ >= 0) \
                                        or (iclass == 2 and di_ <= 0)
                                    if not ok:
                                        continue
                                    for dw in range(3):
                                        tt = s + 1 - dw
                                        dj_ = int(np.floor(tt / 2))
                                        if 0 <= jc + dj_ < 32:
                                            acc += BB3[dh, dw, c4]
                                B[ones, col] = acc
            return B

        w1r1 = w_block(1, [1])
        wm1r0 = w_block(-1, [0])
        blocks[("cat", t)] = np.concatenate([w1r1, w_block(0, [0, 1], 0), wm1r0], axis=1)
        blocks[("e0", t)] = np.concatenate([w_block(0, [0, 1], 1), wm1r0], axis=1)
        blocks[("e24", t)] = w_block(0, [0, 1], 2)
    return blocks


class _Pack:
    def __init__(self):
        self.cols = 0
        self.reg = {}
        self.items = []

    def add(self, key, arr):
        K, M = arr.shape
        self.reg[key] = (self.cols, K, M)
        self.items.append(arr)
        self.cols += M

    def build(self):
        out = np.zeros((128, self.cols), np.float32)
        c = 0
        for arr in self.items:
            K, M = arr.shape
            out[:K, c:c + M] = arr
            c += M
        return out


def _make_packs(inputs):
    P = _precompute(inputs)
    f1b, f1bias = _fused1_blocks(P)
    f2b = _fused2_blocks(P)
    f3b = _fused3_blocks(P)

    # order: lin + bias rows first so the first two 512-col chunks unblock lin
    wp = _Pack()
    wp.add("lin0", P["lhs_lin"][:, 0:128])
    wp.add("lin1", P["lhs_lin"][:, 128:256])
    # row-form biases + a ones row: biases enter PSUM via a K=1 matmul
    wp.add("ones512", np.ones((1, BCORE), np.float32))
    wp.add("rblin0", P["c_lin"][0:128].reshape(1, -1))
    wp.add("rblin1", P["c_lin"][128:256].reshape(1, -1))
    for g in range(3):
        for a in range(4):
            wp.add(("rb1", g, a), f1bias[(g, a)].reshape(1, -1))
    for g in range(3):
        for a in range(4):
            wp.add(("f1", g, a), f1b[(g, a)])

    wb = _Pack()
    for a in range(4):
        for key in [("r12", a), ("m", "mid", a), ("m", "edge", a), ("r0m1", a),
                    ("r4", "mid", a), ("r4", "edge", a), ("r4p1", a)]:
            wb.add(key, f2b[key])
    for t in range(4):
        for key in [("cat", t), ("e0", t), ("e24", t)]:
            wb.add(key, f3b[key])

    bp = _Pack()
    bp.add("blin0", P["c_lin"][0:128].reshape(-1, 1))
    bp.add("blin1", P["c_lin"][128:256].reshape(-1, 1))
    for g in range(3):
        for a in range(4):
            bp.add(("b1", g, a), f1bias[(g, a)].reshape(-1, 1))
    return wp, bp, wb


# ---------------- device program ----------------
_PROG = {}


def _lim(s):
    if s == 0:
        return 128
    if s == 64:
        return 64
    return 32


def _pieces(p0, d0, n):
    assert p0 % 32 == 0 and d0 % 32 == 0, (p0, d0, n)
    out = []
    off = 0
    while off < n:
        s1, s2 = (p0 + off) % 128, (d0 + off) % 128
        c = min(n - off, _lim(s1), _lim(s2))
        out.append((off, c))
        off += c
    return out


def _build_program(wcols, bcols, wbcols):
    key = (wcols, bcols, wbcols)
    if key in _PROG:
        return _PROG[key]
    nc = bacc.Bacc("TRN2", target_bir_lowering=False, debug=False, num_devices=NCORES)
    lat_ap = nc.dram_tensor("latent", [BCORE, 7], F32, kind="ExternalInput").ap()
    wp_ap = nc.dram_tensor("wpack", [128, wcols], F32, kind="ExternalInput").ap()
    bp_ap = nc.dram_tensor("bpack", [128, bcols], F32, kind="ExternalInput").ap()
    wb_ap = nc.dram_tensor("wbpack", [128, wbcols], BF16, kind="ExternalInput").ap()
    # h-major output (host transposes to NCHW): fully contiguous stg DMA
    out_ap = nc.dram_tensor("out", [BCORE, 50, 6, 64], BF16, kind="ExternalOutput").ap()
    ones_ap = nc.dram_tensor("ones", [1, 25 * BCORE], BF16, kind="ExternalInput").ap()
    with tile.TileContext(nc) as tc:
        with ExitStack() as ctx:
            _emit(ctx, tc, nc, lat_ap, wp_ap, bp_ap, wb_ap, out_ap, ones_ap,
                  _build_program.wreg, _build_program.breg, _build_program.wbreg)
    _dedup_ldweights(nc)
    nc.compile()
    _PROG[key] = nc
    return nc


def _dedup_ldweights(nc):
    """Drop InstLdweights whose stationary AP matches the previous load on the
    PE queue (the PE array keeps the stationary across matmuls)."""
    from concourse import mybir
    removed = 0
    for fn in nc.m.functions:
        for blk in fn.blocks:
            insts = list(blk.instructions)
            keep = []
            prev_sig = None
            for ins in insts:
                tn = type(ins).__name__
                if tn == "InstLdweights":
                    sig = (str(ins.ins[0]), str(getattr(ins, "perf_mode", None)),
                           str(getattr(ins, "is_transpose", None)))
                    if sig == prev_sig and not ins.has_wait() and not ins.has_update():
                        removed += 1
                        continue
                    prev_sig = sig
                elif tn == "InstMatmult":
                    pass  # streaming doesn't clobber the loaded stationary
                elif getattr(ins, "engine", None) == mybir.EngineType.PE \
                        and tn not in ("InstEventSemaphore",):
                    prev_sig = None
                keep.append(ins)
            if len(keep) != len(insts):
                blk.instructions = keep
    return removed


def _emit(ctx, tc, nc, lat_ap, wp_ap, bp_ap, wb_ap, out_ap, ones_ap, wreg, breg, wbreg):
    wcols = wp_ap.shape[1]
    bcols = bp_ap.shape[1]
    wbcols = wb_ap.shape[1]

    consts = ctx.enter_context(tc.tile_pool(name="consts", bufs=1))
    x3p = ctx.enter_context(tc.tile_pool(name="x3", bufs=1))
    a3p = ctx.enter_context(tc.tile_pool(name="a3", bufs=1))
    # pools freed before the stg pool opens (wp_r/bounce/x1/lat die after f1)
    early_ctx = ExitStack()
    earlyp = early_ctx.enter_context(tc.tile_pool(name="early", bufs=1))
    bounce = early_ctx.enter_context(tc.tile_pool(name="bounce", bufs=2))
    x1p = early_ctx.enter_context(tc.tile_pool(name="x1", bufs=1))
    # fused1/2 psum pool (single banks), closed before fused3's wide pool
    ps_ctx = ExitStack()
    psp = ps_ctx.enter_context(tc.tile_pool(name="ps", bufs=8, space="PSUM"))

    # ---- constants (lin dependencies first, big fused-weight pack last) ----
    lat_f = earlyp.tile([7, BCORE], F32)
    nc.sync.dma_start(lat_f[:], lat_ap[:].rearrange("b d -> d b"))
    lat_r = earlyp.tile([7, BCORE], F32R)
    nc.vector.tensor_copy(lat_r[:], lat_f[:])
    wp_r = earlyp.tile([128, wcols], F32R)
    for c0 in range(0, wcols, 512):
        n = min(512, wcols - c0)
        bt = bounce.tile([128, 512], F32, tag="bounce", name=f"bw{c0}")
        nc.sync.dma_start(bt[:, :n], wp_ap[:, c0:c0 + n])
        nc.vector.tensor_copy(wp_r[:, c0:c0 + n], bt[:, :n])
    wbt = consts.tile([128, wbcols], BF16)
    nc.sync.dma_start(wbt[:], wb_ap[:])
    bpt = consts.tile([128, bcols], F32)
    nc.sync.dma_start(bpt[:], bp_ap[:])

    def W(key):
        o, K, M = wreg[key]
        return wp_r[:K, o:o + M]

    def WB(key, c0=None, c1=None):
        o, K, M = wbreg[key]
        if c0 is None:
            return wbt[:K, o:o + M]
        return wbt[:K, o + c0:o + c1]

    def BV(key, p0, n):
        o, K, M = breg[key]
        return bpt[p0:p0 + n, o:o + 1]

    # evac engines: ACT 1-pass lrelu; DVE must 2-pass (copy + in-place stt).
    # Greedy balance on estimated per-instruction cost.
    ev_load = [0.0, 0.0]  # ACT, DVE accumulated ns

    def ev_lrelu(dst, src):
        free = src.free_size()
        cost_a = free * 0.833 + 300.0
        cost_d = free * 2.08 + 380.0
        if ev_load[0] + cost_a <= ev_load[1] + cost_d:
            ev_load[0] += cost_a
            nc.scalar.activation(dst, src, AF.Lrelu, bias=0.0, scale=1.0, alpha=0.01)
        else:
            ev_load[1] += cost_d
            nc.vector.tensor_copy(dst, src)
            nc.vector.scalar_tensor_tensor(dst, dst, 0.01, dst, op0=OP.mult, op1=OP.max)

    # ---- x3 / a3 tiles + ones rows ----
    x3t = [x3p.tile([X2_WIN[a][1] * 16 + 1, 5 * BCORE], BF16, tag=f"x3_{a}",
                    name=f"x3_{a}") for a in range(4)]
    a3t = [a3p.tile([F3_WIN[t][1] * 8 + 1, 25 * BCORE], BF16, tag=f"a3_{t}",
                    name=f"a3_{t}") for t in range(4)]
    # ones rows via DMA from a DRAM constant (gpsimd memset is ~10us per row)
    for a in range(4):
        o = _x3_ones(a)
        nc.sync.dma_start(x3t[a][o:o + 1, :], ones_ap[0:1, 0:5 * BCORE])
    for t in range(4):
        o = _a3_ones(t)
        nc.sync.dma_start(a3t[t][o:o + 1, :], ones_ap[0:1, :])

    # ---- lin (bias preloaded into PSUM via K=1 matmul against a ones row) ----
    psA = psp.tile([128, BCORE], F32, tag="ps", name="linA")
    nc.tensor.matmul(psA[:], W("rblin0"), W("ones512"), start=True, stop=False,
                     skip_group_check=True)
    nc.tensor.matmul(psA[:], W("lin0"), lat_r[:], start=False, stop=True,
                     skip_group_check=True)
    psB = psp.tile([128, BCORE], F32, tag="ps", name="linB")
    nc.tensor.matmul(psB[:], W("rblin1"), W("ones512"), start=True, stop=False,
                     skip_group_check=True)
    nc.tensor.matmul(psB[:], W("lin1"), lat_r[:], start=False, stop=True,
                     skip_group_check=True)

    x1t = [x1p.tile([X1_WIN[a][1] * 32, BCORE], F32R, tag=f"x1_{a}", name=f"x1_{a}")
           for a in range(4)]
    for a in range(4):
        wi0, nwi = X1_WIN[a]
        for ps, base in ((psA, 0), (psB, 4)):
            lo = max(wi0, base)
            hi = min(wi0 + nwi, base + 4)
            if lo >= hi:
                continue
            d0 = (lo - wi0) * 32
            p0 = (lo - base) * 32
            n = (hi - lo) * 32
            for off, cnt in _pieces(p0, d0, n):
                ev_lrelu(x1t[a][d0 + off:d0 + off + cnt, :],
                         ps[p0 + off:p0 + off + cnt, :])

    # ---- fused1 (a-outer so x3 mirrors can fire early) ----
    # x3 mirrors: (dst_a, d0, src_a, s0, n)
    X3_MIR = [(0, 64, 1, 0, 16), (1, 64, 0, 32, 32), (1, 96, 2, 0, 16),
              (2, 64, 1, 32, 32), (2, 96, 3, 0, 16), (3, 64, 2, 32, 32)]
    for a in range(4):
        for g, (h0, nh) in enumerate(HG):
            M = nh * 64
            ps = psp.tile([128, BCORE], F32, tag="ps", name=f"f1_{g}_{a}")
            nc.tensor.matmul(ps[0:M, :], W(("rb1", g, a)), W("ones512"),
                             start=True, stop=False, skip_group_check=True)
            nc.tensor.matmul(ps[0:M, :], W(("f1", g, a)), x1t[a][:],
                             start=False, stop=True, skip_group_check=True)
            for hi_ in range(nh):
                hh = h0 + hi_
                # primary region of window a: rows [0:64) = W1 4a..4a+4
                ev_lrelu(x3t[a][0:64, hh * BCORE:(hh + 1) * BCORE],
                         ps[hi_ * 64:hi_ * 64 + 64, :])
        for dst_a, d0, src_a, s0, n in X3_MIR:
            if src_a == a:
                nc.sync.dma_start(x3t[dst_a][d0:d0 + n, :], x3t[a][s0:s0 + n, :])
    early_ctx.close()  # frees wp_r/bounce/x1/lat SBUF for the stg pool

    # ---- fused2 (a-outer so a3 mirrors can fire early) ----
    # a3 mirrors: (dst_t, d0, src_t, s0, n)
    A3_MIR = [(0, 64, 1, 0, 8), (1, 64, 0, 32, 32), (1, 96, 2, 0, 8),
              (2, 64, 1, 32, 32), (2, 96, 3, 0, 8), (3, 64, 2, 32, 32)]

    def xsl(a, i):
        K = X2_WIN[a][1] * 16 + 1
        return x3t[a][0:K, i * BCORE:(i + 1) * BCORE]

    def f2ev1(a, ps, p0, i, r):
        H = 5 * i + r
        ev_lrelu(a3t[a][0:64, H * BCORE:(H + 1) * BCORE], ps[p0:p0 + 64, :])

    for a in range(4):
        # block-outer so consecutive matmuls share their stationary (LW dedup)
        b1 = [psp.tile([128, BCORE], F32, tag="ps", name=f"f2a_{a}_{i}")
              for i in range(5)]
        for i in range(5):
            nc.tensor.matmul(b1[i][:], WB(("r12", a)), xsl(a, i),
                             start=True, stop=True)
            f2ev1(a, b1[i], 0, i, 1)
            f2ev1(a, b1[i], 64, i, 2)
        b2 = [psp.tile([128, BCORE], F32, tag="ps", name=f"f2b_{a}_{i}")
              for i in range(5)]
        nc.tensor.matmul(b2[0][:], WB(("m", "edge", a)), xsl(a, 0),
                         start=True, stop=True, skip_group_check=True)
        f2ev1(a, b2[0], 0, 0, 0)
        f2ev1(a, b2[0], 64, 0, 3)
        for i in range(1, 5):
            nc.tensor.matmul(b2[i][:], WB(("m", "mid", a)), xsl(a, i),
                             start=True, stop=False, skip_group_check=True)
        for i in range(1, 5):
            nc.tensor.matmul(b2[i][0:64, :], WB(("r0m1", a)), xsl(a, i - 1),
                             start=False, stop=True, skip_group_check=True)
            f2ev1(a, b2[i], 0, i, 0)
            f2ev1(a, b2[i], 64, i, 3)
        b3 = [psp.tile([128, BCORE], F32, tag="ps", name=f"f2c_{a}_{i}")
              for i in range(5)]
        for i in range(4):
            nc.tensor.matmul(b3[i][0:64, :], WB(("r4", "mid", a)), xsl(a, i),
                             start=True, stop=False, skip_group_check=True)
        nc.tensor.matmul(b3[4][0:64, :], WB(("r4", "edge", a)), xsl(a, 4),
                         start=True, stop=True, skip_group_check=True)
        f2ev1(a, b3[4], 0, 4, 4)
        for i in range(4):
            nc.tensor.matmul(b3[i][0:64, :], WB(("r4p1", a)), xsl(a, i + 1),
                             start=False, stop=True, skip_group_check=True)
            f2ev1(a, b3[i], 0, i, 4)
        for dst_t, d0, src_t, s0, n in A3_MIR:
            if src_t == a:
                nc.sync.dma_start(a3t[dst_t][d0:d0 + n, :], a3t[a][s0:s0 + n, :])

    # ---- fused3: merged-MM streams ----
    # bank k = [slot 2k | slot 2k+1], slot = 192 cols (r*96 + c4*16 + wloc).
    # Bank pairs live in 2-bank wide PSUM tiles; one evac covers 8 h-rows.
    ps_ctx.close()
    psW = ctx.enter_context(tc.tile_pool(name="psW", bufs=4, space="PSUM"))
    stgp = ctx.enter_context(tc.tile_pool(name="stg", bufs=8))
    for c in range(4):
        cb = c * 128
        stg = [stgp.tile([128, 3072], BF16, tag="stg", name=f"stg_{c}_{m}")
               for m in range(7)]
        wides = {}
        for t in range(4):
            Kt = F3_WIN[t][1] * 8 + 1

            def st(i):
                return a3t[t][0:Kt, i * BCORE + cb:i * BCORE + cb + 128]

            for m in range(7):
                wides[(t, m)] = psW.tile([128, 1024], F32, tag="ps",
                                         name=f"f3_{c}_{t}_{m}")

            def bk(k, c0, c1):
                return wides[(t, k // 2)][:, (k % 2) * 512 + c0:(k % 2) * 512 + c1]

            def evac_wide(m):
                # stg cols (h, c4, w) h-major: PSUM order (q, slot,r,c4,w) is
                # h-major too, so dst collapses to 3D (p, h*c4 chunk, w)
                src = wides[(t, m)][:].rearrange("p (q b) -> p q b", q=2, b=512)
                src = src[:, :, 0:384] if m < 6 else src[:, 0:1, 0:192]
                nch = 48 if m < 6 else 12
                sv = stg[m][:].rearrange("p (hc w) -> p hc w", hc=48, w=64)
                ev_lrelu(sv[:, 0:nch, 16 * t:16 * t + 16], src)
                if t == 3:
                    hr = 8 if m < 6 else 2
                    dview = out_ap[cb:cb + 128, 8 * m:8 * m + hr, :, :]
                    nc.sync.dma_start(dview, stg[m][:, 0:hr * 384])

            cat = ("cat", t)
            for i in range(25):
                k = i // 2
                if i == 0:
                    nc.tensor.matmul(bk(0, 0, 288), st(i), WB(("e0", t)),
                                     start=True, stop=False, skip_group_check=True)
                elif i % 2 == 1:
                    nc.tensor.matmul(bk(k, 96, 384), st(i), WB(cat, 0, 288),
                                     start=False, stop=False, skip_group_check=True)
                    nc.tensor.matmul(bk(k + 1, 0, 96), st(i),
                                     WB(cat, 288, 384), start=True, stop=False,
                                     skip_group_check=True)
                elif i < 24:
                    nc.tensor.matmul(bk(k - 1, 288, 384), st(i),
                                     WB(cat, 0, 96), start=False, stop=True,
                                     skip_group_check=True)
                    nc.tensor.matmul(bk(k, 0, 288), st(i), WB(cat, 96, 384),
                                     start=False, stop=False, skip_group_check=True)
                    if (k - 1) % 2 == 1:
                        evac_wide((k - 1) // 2)
                else:
                    nc.tensor.matmul(bk(11, 288, 384), st(i),
                                     WB(cat, 0, 96), start=False, stop=True,
                                     skip_group_check=True)
                    nc.tensor.matmul(bk(12, 0, 192), st(i), WB(("e24", t)),
                                     start=False, stop=True, skip_group_check=True)
                    evac_wide(5)
                    evac_wide(6)


def kernel(**inputs):
    inputs = {k: np.asarray(v) for k, v in inputs.items()}
    wp, bp, wb = _make_packs(inputs)
    wpack = wp.build()
    bpack = bp.build()
    import ml_dtypes
    wbpack = wb.build().astype(ml_dtypes.bfloat16)
    _build_program.wreg = wp.reg
    _build_program.breg = bp.reg
    _build_program.wbreg = wb.reg
    nc = _build_program(wpack.shape[1], bpack.shape[1], wbpack.shape[1])

    lat = np.ascontiguousarray(inputs["latent"].astype(np.float32))
    ones = np.ones((1, 25 * BCORE), ml_dtypes.bfloat16)
    in_maps = [
        {"latent": lat[i * BCORE:(i + 1) * BCORE], "wpack": wpack,
         "bpack": bpack, "wbpack": wbpack, "ones": ones}
        for i in range(NCORES)
    ]
    res = run_bass_kernel_spmd(nc, in_maps, core_ids=list(range(NCORES)))
    out = np.concatenate(
        [np.asarray(res.results[i]["out"]).astype(np.float32) for i in range(NCORES)],
        axis=0)
    return np.ascontiguousarray(out.transpose(0, 2, 1, 3))  # [B,50,6,64]->[B,6,50,64]
